# revision 11
# baseline (speedup 1.0000x reference)
"""RNN-T Joint network kernel for Trainium2 (8 NeuronCores, SPMD).

Math (per reference):
    pred_proj = h_pred @ W1[:H]          # [N,U,J]
    enc_proj  = h_enc  @ W1[H:]          # [N,T,J]
    z = pred_proj[:,None] + enc_proj[:,:,None] + b1   # [N,T,U,J]
    x = tanh(z)
    logits = x @ W2 + b2                 # [N,T,U,V]
    out = log_softmax(logits / TEMP, -1)

Sharding: the 1000 (n,t) pairs are split into 8 contiguous shards of 125;
each core handles its shard's full (U, V) lattice.  Weights replicated,
no collectives.  Device-side layout keeps J on SBUF partitions for the
z/tanh stage (so the per-(t,u)-row broadcast adds are per-partition-scalar
ops) and (t,u) rows on PSUM partitions for the big GEMM + softmax (so the
V-axis reduction is a free-axis reduction and output DMA is contiguous).
"""

import sys

for _p in ("/opt/trn_rl_repo",):
    if _p not in sys.path:
        sys.path.insert(0, _p)

import numpy as np

N, T, U, H = 4, 250, 80, 512
J, V = 1024, 2048
TEMP = 1.0
NCORES = 8
TSH = (N * T) // NCORES   # 125 (n,t) pairs per core
ROWS = TSH * U            # 10000 (t,u) rows per core
BLK = 128                 # rows per block (PSUM partition dim)
NBLK = (ROWS + BLK - 1) // BLK
KC = J // 128             # 8 contraction chunks for the big GEMM
HC = H // 128             # 4 contraction chunks for the projections
VB = 512                  # free-dim per matmul (one PSUM bank of fp32)
NVB = V // VB


def _segments(r0, m):
    """Rows [r0, r0+m) of the (t,u) lattice as (off, L, t, u0) runs."""
    segs = []
    r = r0
    while r < r0 + m:
        t, u0 = divmod(r, U)
        L = min(U - u0, r0 + m - r)
        segs.append((r - r0, L, t, u0))
        r += L
    return segs


def _emit(tc, out_ap, henct, hpredt, w1, b1t, w2, b2r):
    """Emit the per-core Tile kernel.

    henct:  [HC, 128, TSH] f32   h_enc shard, transposed (h on partitions)
    hpredt: [HC, 128, U]   f32   h_pred row,  transposed
    w1:     [2H, J] f32          (rows: pred half then enc half)
    b1t:    [128, KC] f32        b1 with J split across partitions
    w2:     [J, V] f32
    b2r:    [1, V] f32
    out_ap: [ROWS, V] f32
    """
    import concourse.mybir as mybir
    from contextlib import ExitStack

    nc = tc.nc
    f32 = mybir.dt.float32
    f32r = mybir.dt.float32r
    bf16 = mybir.dt.bfloat16
    AF = mybir.ActivationFunctionType
    ALU = mybir.AluOpType

    with ExitStack() as ctx:
        wpool = ctx.enter_context(tc.tile_pool(name="wpool", bufs=1))

        # Persistent SBUF residents.  Matmul operands must be *produced* as
        # fp32r (the BIR verifier rejects bitcasts of raw fp32), so W2/b2 are
        # staged through fp32 tiles and rounded by a DVE copy once.
        w2_sb = []
        enct = wpool.tile([128, KC, TSH], f32, name="enct")
        predb = wpool.tile([128, KC, U], f32, name="predb")
        b1t_sb = wpool.tile([128, KC], f32, name="b1t_sb")
        nc.sync.dma_start(b1t_sb[:], b1t[:, :])
        b2_sb = wpool.tile([1, V], f32r, name="b2_sb")
        ones_sb = wpool.tile([1, 128], f32r, name="ones_sb")

        # ---- Preamble: pred_projT (+b1) and enc_projT, J on partitions ----
        with tc.tile_pool(name="pre", bufs=1) as pre, \
             tc.tile_pool(name="w2stage", bufs=2) as w2stage, \
             tc.tile_pool(name="prepsum", bufs=2, space="PSUM") as ppsum:
            for k in range(KC):
                w2st = w2stage.tile([128, V], f32, name="w2st")
                nc.sync.dma_start(w2st[:], w2[k * 128:(k + 1) * 128, :])
                w2t = wpool.tile([128, V], f32r, name=f"w2sb{k}")
                nc.vector.tensor_copy(w2t[:], w2st[:])
                w2_sb.append(w2t)
            b2st = w2stage.tile([1, V], f32, name="b2st", bufs=1)
            nc.sync.dma_start(b2st[:], b2r[:, :])
            nc.vector.tensor_copy(b2_sb[:], b2st[:])
            ones_st = w2stage.tile([1, 128], f32, name="ones_st", bufs=1)
            nc.vector.memset(ones_st[:], 1.0)
            nc.vector.tensor_copy(ones_sb[:], ones_st[:])
            w1_sb = []
            for hc in range(2 * HC):
                w1t = pre.tile([128, J], f32, name=f"w1sb{hc}")
                nc.sync.dma_start(w1t[:], w1[hc * 128:(hc + 1) * 128, :])
                w1_sb.append(w1t)
            hpred_sb = []
            henc_sb = []
            for hc in range(HC):
                hpt = pre.tile([128, U], f32, name=f"hpred{hc}")
                nc.sync.dma_start(hpt[:], hpredt[hc, :, :])
                hpred_sb.append(hpt)
                het = pre.tile([128, TSH], f32, name=f"henc{hc}")
                nc.sync.dma_start(het[:], henct[hc, :, :])
                henc_sb.append(het)

            for jc in range(KC):
                js = slice(jc * 128, (jc + 1) * 128)
                pp = ppsum.tile([128, U], f32, name="pp")
                for hc in range(HC):
                    nc.tensor.matmul(
                        pp[:], w1_sb[hc][:, js], hpred_sb[hc][:],
                        start=(hc == 0), stop=(hc == HC - 1))
                # evacuate + bias in one DVE op
                nc.vector.tensor_scalar_add(
                    predb[:, jc, :], pp[:], b1t_sb[:, jc:jc + 1])
                ep = ppsum.tile([128, TSH], f32, name="ep")
                for hc in range(HC):
                    nc.tensor.matmul(
                        ep[:], w1_sb[HC + hc][:, js], henc_sb[hc][:],
                        start=(hc == 0), stop=(hc == HC - 1))
                nc.vector.tensor_copy(enct[:, jc, :], ep[:])

        # ---- Main loop over 128-row blocks of the (t,u) lattice ----
        zpool = ctx.enter_context(tc.tile_pool(name="zpool", bufs=2))
        xpool = ctx.enter_context(tc.tile_pool(name="xpool", bufs=2))
        epool = ctx.enter_context(tc.tile_pool(name="epool", bufs=1))
        opool = ctx.enter_context(tc.tile_pool(name="opool", bufs=3))
        spool = ctx.enter_context(tc.tile_pool(name="spool", bufs=2))
        mpsum = ctx.enter_context(tc.tile_pool(name="mpsum", bufs=2, space="PSUM"))

        inv_temp = 1.0 / TEMP
        for b in range(NBLK):
            r0 = b * BLK
            m = min(BLK, ROWS - r0)
            segs = _segments(r0, m)

            # z^T for this block: [J-chunk partitions, (k, row)] = pred + enc + b1
            zt = zpool.tile([128, KC, BLK], f32, name="zt")
            for k in range(KC):
                for (off, L, t, u0) in segs:
                    nc.vector.tensor_scalar_add(
                        zt[:, k, off:off + L],
                        predb[:, k, u0:u0 + L],
                        enct[:, k, t:t + 1])
            xt = xpool.tile([128, KC, BLK], f32r, name="xt")
            nc.scalar.activation(xt[:, :, :m], zt[:, :, :m], AF.Tanh)

            # logits block: [m rows, V] in PSUM; accumulate over KC chunks,
            # then add b2 via a rank-1 ones x b2 matmul.
            ps = mpsum.tile([128, V], f32, name="ps")
            for k in range(KC):
                for v in range(NVB):
                    nc.tensor.matmul(
                        ps[:m, v * VB:(v + 1) * VB],
                        xt[:, k, :m],
                        w2_sb[k][:, v * VB:(v + 1) * VB],
                        start=(k == 0), stop=False)
            for v in range(NVB):
                nc.tensor.matmul(
                    ps[:m, v * VB:(v + 1) * VB],
                    ones_sb[:, :m],
                    b2_sb[:, v * VB:(v + 1) * VB],
                    start=False, stop=True)

            # logsumexp over V (free axis): exp with fused row-sum, then ln.
            et = epool.tile([128, V], bf16, name="et")
            sums = spool.tile([128, 1], f32, name="sums")
            nc.scalar.activation(et[:m, :], ps[:m, :], AF.Exp,
                                 scale=inv_temp, accum_out=sums[:m, :])
            lse = spool.tile([128, 1], f32, name="lse")
            nc.scalar.activation(lse[:m, :], sums[:m, :], AF.Ln)

            # out = logits/TEMP - lse, straight from PSUM to SBUF to HBM.
            ot = opool.tile([128, V], f32, name="ot")
            nc.vector.tensor_scalar(
                ot[:m, :], ps[:m, :], inv_temp, lse[:m, :1],
                op0=ALU.mult, op1=ALU.subtract)
            nc.sync.dma_start(out_ap[r0:r0 + m, :], ot[:m, :])


_CACHE = {}


def _build():
    if "nc" in _CACHE:
        return _CACHE["nc"]
    from concourse import bacc
    import concourse.tile as tile
    import concourse.mybir as mybir

    nc = bacc.Bacc("TRN2", target_bir_lowering=False, debug=False,
                   enable_asserts=False, num_devices=NCORES)
    f32 = mybir.dt.float32
    henct = nc.dram_tensor("henct", (HC, 128, TSH), f32, kind="ExternalInput").ap()
    hpredt = nc.dram_tensor("hpredt", (HC, 128, U), f32, kind="ExternalInput").ap()
    w1 = nc.dram_tensor("w1", (2 * H, J), f32, kind="ExternalInput").ap()
    b1t = nc.dram_tensor("b1t", (128, KC), f32, kind="ExternalInput").ap()
    w2 = nc.dram_tensor("w2", (J, V), f32, kind="ExternalInput").ap()
    b2r = nc.dram_tensor("b2r", (1, V), f32, kind="ExternalInput").ap()
    out = nc.dram_tensor("out", (ROWS, V), f32, kind="ExternalOutput").ap()

    with tile.TileContext(nc) as tc:
        _emit(tc, out, henct, hpredt, w1, b1t, w2, b2r)
    nc.compile()
    _CACHE["nc"] = nc
    return nc


def _in_maps(h_pred, h_enc, W1, b1, W2, b2):
    w1 = np.ascontiguousarray(W1, np.float32)
    b1t = np.ascontiguousarray(b1.astype(np.float32).reshape(KC, 128).T)
    w2 = np.ascontiguousarray(W2, np.float32)
    b2r = np.ascontiguousarray(b2, np.float32).reshape(1, V)
    maps = []
    for c in range(NCORES):
        n, t0 = divmod(c * TSH, T)
        henct = np.ascontiguousarray(
            h_enc[n, t0:t0 + TSH, :].T, np.float32).reshape(HC, 128, TSH)
        hpredt = np.ascontiguousarray(
            h_pred[n].T, np.float32).reshape(HC, 128, U)
        maps.append({"henct": henct, "hpredt": hpredt, "w1": w1,
                     "b1t": b1t, "w2": w2, "b2r": b2r})
    return maps


def _make_pjrt_fn(nc):
    """Mirror bass2jax.run_bass_via_pjrt, but return a reusable jitted fn."""
    import jax
    import concourse.mybir as mybir
    from concourse import bass2jax
    from jax.experimental.shard_map import shard_map
    from jax.sharding import Mesh, PartitionSpec

    bass2jax.install_neuronx_cc_hook()
    partition_name = (nc.partition_id_tensor.name
                      if nc.partition_id_tensor else None)
    in_names, out_names, out_avals, zero_outs = [], [], [], []
    for alloc in nc.m.functions[0].allocations:
        if not isinstance(alloc, mybir.MemoryLocationSet):
            continue
        name = alloc.memorylocations[0].name
        if alloc.kind == "ExternalInput":
            if name != partition_name:
                in_names.append(name)
        elif alloc.kind == "ExternalOutput":
            out_names.append(name)
            shape = tuple(alloc.tensor_shape)
            dtype = mybir.dt.np(alloc.dtype)
            out_avals.append(jax.core.ShapedArray(shape, dtype))
            zero_outs.append(np.zeros(shape, dtype))
    n_params = len(in_names)
    all_in_names = in_names + out_names
    if partition_name is not None:
        all_in_names = all_in_names + [partition_name]

    def _body(*args):
        operands = list(args)
        if partition_name is not None:
            operands.append(bass2jax.partition_id_tensor())
        outs = bass2jax._bass_exec_p.bind(
            *operands,
            out_avals=tuple(out_avals),
            in_names=tuple(all_in_names),
            out_names=tuple(out_names),
            lowering_input_output_aliases=(),
            sim_require_finite=True,
            sim_require_nnan=True,
            nc=nc,
        )
        return tuple(outs)

    devices = jax.devices()[:NCORES]
    mesh = Mesh(np.asarray(devices), ("core",))
    spec = PartitionSpec("core")
    fn = jax.jit(
        shard_map(_body, mesh=mesh,
                  in_specs=(spec,) * (n_params + len(out_names)),
                  out_specs=(spec,) * len(out_names),
                  check_rep=False),
        keep_unused=True)
    sharding = jax.sharding.NamedSharding(mesh, spec)
    return fn, in_names, out_names, out_avals, zero_outs, sharding


def _compiled():
    if "fn" in _CACHE:
        return _CACHE["fn"]
    nc = _build()
    _CACHE["fn"] = _make_pjrt_fn(nc)
    return _CACHE["fn"]


def _device_args(maps):
    import jax
    fn, in_names, out_names, out_avals, zero_outs, sharding = _compiled()
    concat_in = [np.concatenate([m[name] for m in maps], axis=0)
                 for name in in_names]
    concat_zeros = [np.zeros((NCORES * z.shape[0], *z.shape[1:]), z.dtype)
                    for z in zero_outs]
    return [jax.device_put(a, sharding) for a in (concat_in + concat_zeros)]


def kernel(h_pred, h_enc, W1, b1, W2, b2):
    import jax
    maps = _in_maps(np.asarray(h_pred), np.asarray(h_enc), np.asarray(W1),
                    np.asarray(b1), np.asarray(W2), np.asarray(b2))
    fn = _compiled()[0]
    args = _device_args(maps)
    out_arr = jax.block_until_ready(fn(*args))[0]
    out_np = np.asarray(out_arr).reshape(NCORES, ROWS, V)
    out = np.empty((N, T, U, V), np.float32)
    for c in range(NCORES):
        n, t0 = divmod(c * TSH, T)
        out[n, t0:t0 + TSH] = out_np[c].reshape(TSH, U, V)
    return out


def bench(maps, iters=30):
    """Per-call latencies (s) of the kernel with device-resident args."""
    import jax, time
    fn = _compiled()[0]
    args = _device_args(maps)
    jax.block_until_ready(fn(*args))  # warmup
    times = []
    for _ in range(iters):
        t0 = time.perf_counter()
        jax.block_until_ready(fn(*args))
        times.append(time.perf_counter() - t0)
    return times


def bench_null(iters=30):
    """Per-call latencies (s) of a trivial NEFF — axon RPC/dispatch floor."""
    import jax, time
    if "null" not in _CACHE:
        from concourse import bacc
        import concourse.tile as tile
        import concourse.mybir as mybir
        nc = bacc.Bacc("TRN2", target_bir_lowering=False, debug=False,
                       enable_asserts=False, num_devices=NCORES)
        f32 = mybir.dt.float32
        xin = nc.dram_tensor("xin", (128, 4), f32, kind="ExternalInput").ap()
        xout = nc.dram_tensor("xout", (128, 4), f32, kind="ExternalOutput").ap()
        with tile.TileContext(nc) as tc:
            with tc.tile_pool(name="p", bufs=1) as pool:
                t = pool.tile([128, 4], f32, name="t")
                nc.sync.dma_start(t[:], xin[:, :])
                nc.sync.dma_start(xout[:, :], t[:])
        nc.compile()
        _CACHE["null"] = _make_pjrt_fn(nc)
    import jax as _jax
    fn, in_names, out_names, out_avals, zero_outs, sharding = _CACHE["null"]
    args = [_jax.device_put(np.zeros((NCORES * 128, 4), np.float32), sharding)
            for _ in range(2)]
    _jax.block_until_ready(fn(*args))
    times = []
    for _ in range(iters):
        t0 = time.perf_counter()
        _jax.block_until_ready(fn(*args))
        times.append(time.perf_counter() - t0)
    return times
